# revision 5
# baseline (speedup 1.0000x reference)
"""Single-head attention on Trainium2: out = softmax(x Wq (x Wk)^T / sqrt(64)) (x Wv).

Full inputs: x [8, 2048, 512], Wq/Wk/Wv [512, 64]. Data-parallel over batch:
core b computes batch element b. Per core (cost-model-driven schedule, v2):

  - ACT-bound design: ACT does ONLY the 32 [128,1024] exps (33.2us floor);
    every other engine hides behind that window.
  - bf16 operands everywhere past the x transposes (matmul cost is 1 cyc/row
    at any free size in bf16; f32r pays 4x below 256-free).
  - PV runs in NATURAL orientation: stationary = pT [128k x 128q] slices,
    moving = v_sb [128k, 65] (ones column appended -> denominators land in
    column 64 of the accumulator). 8 matmuls of 65-free per pair (27ns each)
    instead of 2 of 512-free: PE cost halves and the entire transpose-back
    tail disappears.
  - v is projected directly in natural [s, d] form (stationary = xT chunks,
    moving = Wv, 64-free matmuls).
  - q/k projections packed [Wq|Wk] for all 4 groups; kT lives on partitions
    64-127, q mirrored there by a per-group SBUF DMA.
  - PSUM: 4 banks sT double-buffer ("a"), 2 one-bank accumulators rotating
    between groups (4 q-tile regions per bank via lazy pending-zero
    semantics: start=True only on the very first region write), 2 staging
    banks for in-loop transposes/projections (+ the accumulator banks are
    borrowed for staging before their first PV pop).
  - Per-group flush: reciprocal of the ones column + 4 tensor_scalar_muls +
    one DMA out; no PE involvement.
"""

import numpy as np

B, S, E, D = 8, 2048, 512, 64
NCORES = 8
NT = S // 128   # 16 s-tiles
NE = E // 128   # 4 e-chunks
NG = 4          # row groups of 512 (4 s-tiles each)
SCALE = 1.0 / float(np.sqrt(D))
N_WARM = 10

_CACHE = {}


def _build():
    import concourse.bass as bass
    import concourse.tile as tile
    from concourse import bacc, mybir
    from concourse.masks import make_identity

    f32 = mybir.dt.float32
    f32r = mybir.dt.float32r
    bf16 = mybir.dt.bfloat16
    AF = mybir.ActivationFunctionType

    nc = bacc.Bacc("TRN2", target_bir_lowering=False, debug=False,
                   num_devices=NCORES)

    x_d = nc.dram_tensor("x", [S, E], f32r, kind="ExternalInput").ap()
    wq_d = nc.dram_tensor("Wq", [E, D], f32r, kind="ExternalInput").ap()
    wk_d = nc.dram_tensor("Wk", [E, D], f32r, kind="ExternalInput").ap()
    wv_d = nc.dram_tensor("Wv", [E, D], f32r, kind="ExternalInput").ap()
    out_d = nc.dram_tensor("out", [S, D], f32, kind="ExternalOutput").ap()

    with tile.TileContext(nc) as tc:
        with (
            tc.tile_pool(name="persist", bufs=1) as pp,
            tc.tile_pool(name="ptp", bufs=12) as ptp,
            tc.tile_pool(name="small", bufs=4) as sp,
            tc.tile_pool(name="ps", bufs=1, space="PSUM") as ps,
        ):
            ident = pp.tile([128, 128], f32)
            make_identity(nc, ident[:])
            identr = pp.tile([128, 128], f32r)
            nc.vector.tensor_copy(identr[:], ident[:])

            wqk_s = pp.tile([128, NE, 128], bf16)   # q cols 0:64 | k cols 64:128
            wv_s = pp.tile([128, NE, D], bf16)

            # x: first four s-tiles individually (fast start), then pairs
            x_r = x_d.rearrange("(t p) e -> p t e", p=128)
            x_c = {}
            for t in range(4):
                xc = pp.tile([128, E], f32r, name=f"x_s{t}", tag=f"x_s{t}")
                nc.sync.dma_start(xc[:], x_r[:, t, :])
                x_c[t] = xc
            wq_tmp = pp.tile([128, NE * D], f32r, name="wq_tmp", tag="wq_tmp")
            wk_tmp = pp.tile([128, NE * D], f32r, name="wk_tmp", tag="wk_tmp")
            nc.sync.dma_start(
                wq_tmp[:], wq_d.rearrange("(p a) d -> p (a d)", a=NE))
            nc.sync.dma_start(
                wk_tmp[:], wk_d.rearrange("(p a) d -> p (a d)", a=NE))
            xp = {}
            for pr in range(2, 8):
                xc = pp.tile([128, 2, E], f32r, name=f"x_p{pr}", tag=f"x_p{pr}")
                nc.sync.dma_start(xc[:], x_r[:, 2 * pr:2 * pr + 2, :])
                xp[pr] = xc
                for i in range(2):
                    x_c[2 * pr + i] = None  # resolved via xp below
                if pr == 2:
                    wv_tmp = pp.tile([128, NE * D], f32r, name="wv_tmp",
                                     tag="wv_tmp")
                    nc.sync.dma_start(
                        wv_tmp[:], wv_d.rearrange("(p a) d -> p (a d)", a=NE))

            def xsrc(t):
                if t < 4:
                    return x_c[t][:]
                return xp[t // 2][:, t % 2, :]

            # preload exp ACT table off the critical path
            dummy = sp.tile([128, 1], f32, name="dummy")
            nc.scalar.activation(dummy[:], ident[:, 0:1], AF.Exp)
            # W casts to bf16 (pre-loop: ACT is free here)
            nc.scalar.copy(wqk_s[:, :, 0:D],
                           wq_tmp.rearrange("p (a d) -> p a d", a=NE))
            nc.vector.tensor_copy(wqk_s[:, :, D:128],
                                  wk_tmp.rearrange("p (a d) -> p a d", a=NE))
            nc.vector.tensor_copy(wv_s[:, :, :],
                                  wv_tmp.rearrange("p (a d) -> p a d", a=NE))

            xT_g = [pp.tile([128, NE, 512], bf16, name=f"xT_g{g}",
                            tag=f"xT_g{g}") for g in range(NG)]
            qkT_g = [pp.tile([128, 512], bf16, name=f"qkT_g{g}",
                             tag=f"qkT_g{g}") for g in range(NG)]
            q2_g = [pp.tile([128, 512], bf16, name=f"q2_g{g}",
                            tag=f"q2_g{g}") for g in range(NG)]
            v_sb = pp.tile([128, NT, D + 1], bf16, name="v_sb", tag="v_sb")
            nc.gpsimd.memset(v_sb[:, :, D:D + 1], 1.0)

            warm_tile = ps.tile([128, 1024], f32, tag="a", bufs=2,
                                name="warm")

            def emit_warm(n):
                for i in range(n):
                    j = i % 8
                    nc.tensor.transpose(
                        warm_tile[:, j * 128:(j + 1) * 128], ident[:],
                        ident[:])

            def copy_via(eng, dst, src):
                if eng == "s":
                    nc.scalar.copy(dst, src)
                elif eng == "p":
                    nc.gpsimd.tensor_copy(dst, src)
                else:
                    nc.vector.tensor_copy(dst, src)

            # ---- x^T: one s-tile per PSUM slot-tile ----
            def emit_tp_tile(t, tag, eng, bufs=1):
                g, sub = t // 4, t % 4
                pst = ps.tile([128, 512], f32r, tag=tag, bufs=bufs,
                              name=f"tpt{t}")
                for a in range(NE):
                    nc.tensor.transpose(
                        pst[:, a * 128:(a + 1) * 128],
                        xsrc(t).rearrange("p (ee a) -> p a ee", a=NE)[:, a, :],
                        identr[:],
                    )
                dst = xT_g[g].rearrange(
                    "p a (sp s) -> p a sp s", sp=4)[:, :, sub, :]
                src = pst.rearrange("p (a s) -> p a s", a=NE)
                if eng == "sv":
                    copy_via("s", dst[:, 0:2, :], src[:, 0:2, :])
                    copy_via("v", dst[:, 2:4, :], src[:, 2:4, :])
                else:
                    copy_via(eng, dst, src)

            # ---- packed [Wq|Wk] projection for group g ----
            def emit_proj(g, tag, engs=("v", "v")):
                pj = ps.tile([128, 512], f32, tag=tag, bufs=1, name=f"pj{g}")
                for ec in range(NE):
                    nc.tensor.matmul(
                        pj[:, :], wqk_s[:, ec, :], xT_g[g][:, ec, :],
                        start=(ec == 0), stop=(ec == NE - 1))
                # k half (scores stationary) first, then q half + mirror
                copy_via(engs[0], qkT_g[g][64:128, :], pj[64:128, :])
                copy_via(engs[1], qkT_g[g][0:64, :], pj[0:64, :])
                nc.sync.dma_start(q2_g[g][64:128, :], qkT_g[g][0:64, :])

            # ---- v natural: 4 s-tiles (one quad) per staging bank ----
            def emit_vquad(quad, tag, eng="v"):
                vnp = ps.tile([128, 4 * D], f32, tag=tag, bufs=1,
                              name=f"vq{quad}")
                for i in range(4):
                    st = quad * 4 + i
                    g, sub = st // 4, st % 4
                    for ec in range(NE):
                        nc.tensor.matmul(
                            vnp[:, i * D:(i + 1) * D],
                            xT_g[g][:, ec, sub * 128:(sub + 1) * 128],
                            wv_s[:, ec, :],
                            start=(i == 0 and ec == 0),
                            stop=(ec == NE - 1),
                            skip_group_check=True)
                copy_via(eng,
                         v_sb[:, quad * 4:(quad + 1) * 4, 0:D],
                         vnp.rearrange("p (t c) -> p t c", c=D))

            # ---- main loop ----
            out_r = out_d.rearrange("(t p) d -> p t d", p=128)
            acc = {}
            ACTAG = ["ac0", "ac1"]
            pend = []

            def get_acc(gq):
                if gq not in acc:
                    acc[gq] = ps.tile([128, 512], f32, tag=ACTAG[gq % 2],
                                      bufs=1, name=f"acc{gq}")
                return acc[gq]

            def emit_pv(gq, kt, pT, half):
                a = get_acc(gq)
                for j in range(4):
                    nc.tensor.matmul(
                        a[:, j * (D + 1):j * (D + 1) + D + 1],
                        pT[:, half * 512 + j * 128:half * 512 + (j + 1) * 128],
                        v_sb[:, kt, :],
                        start=(kt == 0 and j == 0),
                        stop=(kt == NT - 1),
                        skip_group_check=True,
                    )

            def emit_pair(gq, ktp, pops, split_exp=False):
                sT = ps.tile([128, 1024], f32, tag="a", bufs=2,
                             name=f"sT{gq}_{ktp}")
                for half in range(2):
                    kt = 2 * ktp + half
                    gk = kt // 4
                    ksl = slice((kt % 4) * 128, (kt % 4 + 1) * 128)
                    nc.tensor.matmul(sT[:, half * 512:(half + 1) * 512],
                                     qkT_g[gk][64:128, ksl],
                                     q2_g[gq][64:128, :],
                                     start=True, stop=True)
                pT = ptp.tile([128, 1024], bf16, name="pT")
                if split_exp:
                    nc.scalar.activation(pT[:, 0:512], sT[:, 0:512],
                                         AF.Exp, scale=SCALE)
                    pend.append((gq, 2 * ktp, pT, 0))
                    for _ in range(min(pops, len(pend))):
                        emit_pv(*pend.pop(0))
                    nc.scalar.activation(pT[:, 512:1024], sT[:, 512:1024],
                                         AF.Exp, scale=SCALE)
                    pend.append((gq, 2 * ktp + 1, pT, 1))
                else:
                    nc.scalar.activation(pT[:], sT[:], AF.Exp, scale=SCALE)
                    pend.append((gq, 2 * ktp, pT, 0))
                    pend.append((gq, 2 * ktp + 1, pT, 1))
                    for _ in range(pops):
                        emit_pv(*pend.pop(0))

            # ---- per-group flush: recip + scale + DMA out ----
            lrec = sp.tile([128, NG, 4], f32, name="lrec", tag="lrec")
            out_sbg = {}

            def emit_flush(gq, engs=("v", "v", "v", "v")):
                a = acc[gq]
                av = a[:, 0:4 * (D + 1)].rearrange(
                    "p (t c) -> p t c", c=D + 1)
                out_sbg[gq] = pp.tile([128, 4, D], f32, name=f"out_sb{gq}",
                                      tag=f"out_sb{gq}")
                nc.vector.reciprocal(lrec[:, gq, 0:4], av[:, :, D])
                for jj in range(4):
                    if engs[jj] == "p":
                        nc.gpsimd.tensor_scalar_mul(
                            out_sbg[gq][:, jj, :], av[:, jj, 0:D],
                            lrec[:, gq, jj:jj + 1])
                    else:
                        nc.vector.tensor_scalar_mul(
                            out_sbg[gq][:, jj, :], av[:, jj, 0:D],
                            lrec[:, gq, jj:jj + 1])
                nc.sync.dma_start(out_r[:, gq * 4:gq * 4 + 4, :],
                                  out_sbg[gq][:, :, :])

            # ================= emission =================
            emit_warm(N_WARM)
            emit_tp_tile(0, "a", "s", bufs=2)
            emit_tp_tile(1, "a", "v", bufs=2)
            emit_tp_tile(2, "a", "s", bufs=2)
            emit_tp_tile(3, "a", "v", bufs=2)
            emit_proj(0, "sg0", engs=("s", "v"))
            emit_tp_tile(4, "ac0", "s")
            emit_tp_tile(5, "ac1", "v")
            emit_tp_tile(6, "sg1", "s")
            emit_tp_tile(7, "sg0", "v")
            emit_vquad(0, "sg1", "v")

            fillers = {
                0: [lambda: emit_proj(1, "sg0")],
                1: [lambda: emit_tp_tile(8, "sg1", "v"),
                    lambda: emit_tp_tile(9, "ac0", "v")],
                2: [lambda: emit_tp_tile(10, "sg0", "v"),
                    lambda: emit_tp_tile(11, "ac1", "v")],
                3: [lambda: emit_proj(2, "sg1"),
                    lambda: emit_vquad(1, "sg0", "v")],
                4: [lambda: emit_tp_tile(12, "sg0", "v")],
                5: [lambda: emit_tp_tile(13, "sg0", "v")],
                6: [lambda: emit_tp_tile(14, "sg1", "v")],
                7: [lambda: emit_tp_tile(15, "sg0", "v"),
                    lambda: emit_vquad(2, "sg1", "v")],
                8: [lambda: emit_proj(3, "sg0")],
                10: [lambda: emit_vquad(3, "sg1", "v")],
            }

            pairs = ([(0, k) for k in range(4)] + [(1, k) for k in range(4)]
                     + [(0, k) for k in range(4, 8)]
                     + [(1, k) for k in range(4, 8)]
                     + [(2, k) for k in range(8)]
                     + [(3, k) for k in range(8)])
            pops_sched = [0, 0, 0] + [2] * 25 + [3, 3, 4, 4]
            flush_at = {14: 0, 18: 1, 26: 2}
            for pi, (gq, ktp) in enumerate(pairs):
                last = (pi == len(pairs) - 1)
                emit_pair(gq, ktp, pops_sched[pi], split_exp=last)
                for f in fillers.get(pi, []):
                    f()
                if pi in flush_at:
                    emit_flush(flush_at[pi])
            while pend:
                emit_pv(*pend.pop(0))
            emit_flush(3)

    nc.compile()
    return nc


def kernel(**inputs):
    from concourse.bass_utils import run_bass_kernel_spmd

    x = np.ascontiguousarray(np.asarray(inputs["x"], dtype=np.float32))
    wq = np.ascontiguousarray(np.asarray(inputs["Wq"], dtype=np.float32))
    wk = np.ascontiguousarray(np.asarray(inputs["Wk"], dtype=np.float32))
    wv = np.ascontiguousarray(np.asarray(inputs["Wv"], dtype=np.float32))

    if "nc" not in _CACHE:
        _CACHE["nc"] = _build()
    nc = _CACHE["nc"]

    in_maps = [
        {"x": np.ascontiguousarray(x[b]), "Wq": wq, "Wk": wk, "Wv": wv}
        for b in range(B)
    ]
    res = run_bass_kernel_spmd(nc, in_maps, core_ids=list(range(NCORES)))
    _CACHE["last_results"] = res
    out = np.stack([res.results[b]["out"] for b in range(B)], axis=0)
    return out


# revision 6
# speedup vs baseline: 1.0629x; 1.0629x over previous
"""Single-head attention on Trainium2: out = softmax(x Wq (x Wk)^T / sqrt(64)) (x Wv).

Full inputs: x [8, 2048, 512], Wq/Wk/Wv [512, 64]. Data-parallel over batch:
core b computes batch element b. Per core (cost-model-driven schedule, v2):

  - ACT-bound design: ACT does ONLY the 32 [128,1024] exps (33.2us floor);
    every other engine hides behind that window.
  - bf16 operands everywhere past the x transposes (matmul cost is 1 cyc/row
    at any free size in bf16; f32r pays 4x below 256-free).
  - PV runs in NATURAL orientation: stationary = pT [128k x 128q] slices,
    moving = v_sb [128k, 65] (ones column appended -> denominators land in
    column 64 of the accumulator). 8 matmuls of 65-free per pair (27ns each)
    instead of 2 of 512-free: PE cost halves and the entire transpose-back
    tail disappears.
  - v is projected directly in natural [s, d] form (stationary = xT chunks,
    moving = Wv, 64-free matmuls).
  - q/k projections packed [Wq|Wk] for all 4 groups; kT lives on partitions
    64-127, q mirrored there by a per-group SBUF DMA.
  - PSUM: 4 banks sT double-buffer ("a"), 2 one-bank accumulators rotating
    between groups (4 q-tile regions per bank via lazy pending-zero
    semantics: start=True only on the very first region write), 2 staging
    banks for in-loop transposes/projections (+ the accumulator banks are
    borrowed for staging before their first PV pop).
  - Per-group flush: reciprocal of the ones column + 4 tensor_scalar_muls +
    one DMA out; no PE involvement.
"""

import numpy as np

B, S, E, D = 8, 2048, 512, 64
NCORES = 8
NT = S // 128   # 16 s-tiles
NE = E // 128   # 4 e-chunks
NG = 4          # row groups of 512 (4 s-tiles each)
SCALE = 1.0 / float(np.sqrt(D))
N_WARM = 10

_CACHE = {}


def _build():
    import concourse.bass as bass
    import concourse.tile as tile
    from concourse import bacc, mybir
    from concourse.masks import make_identity

    f32 = mybir.dt.float32
    f32r = mybir.dt.float32r
    bf16 = mybir.dt.bfloat16
    AF = mybir.ActivationFunctionType

    nc = bacc.Bacc("TRN2", target_bir_lowering=False, debug=False,
                   num_devices=NCORES)

    x_d = nc.dram_tensor("x", [S, E], f32r, kind="ExternalInput").ap()
    wq_d = nc.dram_tensor("Wq", [E, D], f32r, kind="ExternalInput").ap()
    wk_d = nc.dram_tensor("Wk", [E, D], f32r, kind="ExternalInput").ap()
    wv_d = nc.dram_tensor("Wv", [E, D], f32r, kind="ExternalInput").ap()
    out_d = nc.dram_tensor("out", [S, D], f32, kind="ExternalOutput").ap()

    with tile.TileContext(nc) as tc:
        with (
            tc.tile_pool(name="persist", bufs=1) as pp,
            tc.tile_pool(name="ptp", bufs=12) as ptp,
            tc.tile_pool(name="small", bufs=4) as sp,
            tc.tile_pool(name="ps", bufs=1, space="PSUM") as ps,
        ):
            # x: first four s-tiles individually (fast start); pairs s4-7
            # next; s8-15 + wv + mirrors are interleaved into the DMA ring
            # later (mirrors must not queue behind the whole x stream).
            x_r = x_d.rearrange("(t p) e -> p t e", p=128)
            x_c = {}
            for t in range(4):
                xc = pp.tile([128, E], f32r, name=f"x_s{t}", tag=f"x_s{t}")
                nc.sync.dma_start(xc[:], x_r[:, t, :])
                x_c[t] = xc
            wq_tmp = pp.tile([128, NE * D], f32r, name="wq_tmp", tag="wq_tmp")
            wk_tmp = pp.tile([128, NE * D], f32r, name="wk_tmp", tag="wk_tmp")
            nc.sync.dma_start(
                wq_tmp[:], wq_d.rearrange("(p a) d -> p (a d)", a=NE))
            nc.sync.dma_start(
                wk_tmp[:], wk_d.rearrange("(p a) d -> p (a d)", a=NE))
            xp = {}
            for pr in range(2, 8):
                xc = pp.tile([128, 2, E], f32r, name=f"x_p{pr}", tag=f"x_p{pr}")
                xp[pr] = xc
                for i in range(2):
                    x_c[2 * pr + i] = None  # resolved via xp below
            for pr in (2, 3):
                nc.sync.dma_start(xp[pr][:], x_r[:, 2 * pr:2 * pr + 2, :])
            wv_tmp = pp.tile([128, NE * D], f32r, name="wv_tmp", tag="wv_tmp")

            ident = pp.tile([128, 128], f32)
            make_identity(nc, ident[:])
            identr = pp.tile([128, 128], f32r)
            nc.vector.tensor_copy(identr[:], ident[:])

            wqk_s = pp.tile([128, NE, 128], bf16)   # q cols 0:64 | k cols 64:128
            wv_s = pp.tile([128, NE, D], bf16)

            def xsrc(t):
                if t < 4:
                    return x_c[t][:]
                return xp[t // 2][:, t % 2, :]

            # preload exp ACT table off the critical path
            dummy = sp.tile([128, 1], f32, name="dummy")
            nc.scalar.activation(dummy[:], ident[:, 0:1], AF.Exp)
            # W casts to bf16 (pre-loop: ACT is free here)
            nc.scalar.copy(wqk_s[:, :, 0:D],
                           wq_tmp.rearrange("p (a d) -> p a d", a=NE))
            nc.vector.tensor_copy(wqk_s[:, :, D:128],
                                  wk_tmp.rearrange("p (a d) -> p a d", a=NE))

            xT_g = [pp.tile([128, NE, 512], bf16, name=f"xT_g{g}",
                            tag=f"xT_g{g}") for g in range(NG)]
            qkT_g = [pp.tile([128, 512], bf16, name=f"qkT_g{g}",
                             tag=f"qkT_g{g}") for g in range(NG)]
            q2_g = [pp.tile([128, 512], bf16, name=f"q2_g{g}",
                            tag=f"q2_g{g}") for g in range(NG)]
            v_sb = pp.tile([128, NT, D + 1], bf16, name="v_sb", tag="v_sb")
            nc.gpsimd.memset(v_sb[:, :, D:D + 1], 1.0)

            warm_tile = ps.tile([128, 1024], f32, tag="a", bufs=2,
                                name="warm")

            def emit_warm(n):
                for i in range(n):
                    j = i % 8
                    nc.tensor.transpose(
                        warm_tile[:, j * 128:(j + 1) * 128], ident[:],
                        ident[:])

            def copy_via(eng, dst, src):
                if eng == "s":
                    nc.scalar.copy(dst, src)
                elif eng == "p":
                    nc.gpsimd.tensor_copy(dst, src)
                else:
                    nc.vector.tensor_copy(dst, src)

            # ---- x^T: one s-tile per PSUM slot-tile ----
            def emit_tp_tile(t, tag, eng, bufs=1):
                g, sub = t // 4, t % 4
                pst = ps.tile([128, 512], f32r, tag=tag, bufs=bufs,
                              name=f"tpt{t}")
                for a in range(NE):
                    nc.tensor.transpose(
                        pst[:, a * 128:(a + 1) * 128],
                        xsrc(t).rearrange("p (ee a) -> p a ee", a=NE)[:, a, :],
                        identr[:],
                    )
                dst = xT_g[g].rearrange(
                    "p a (sp s) -> p a sp s", sp=4)[:, :, sub, :]
                src = pst.rearrange("p (a s) -> p a s", a=NE)
                if eng == "sv":
                    copy_via("s", dst[:, 0:2, :], src[:, 0:2, :])
                    copy_via("v", dst[:, 2:4, :], src[:, 2:4, :])
                else:
                    copy_via(eng, dst, src)

            # ---- packed [Wq|Wk] projection for group g ----
            def emit_proj(g, tag, engs=("v", "v"), q_first=False,
                          after_mirror=None):
                pj = ps.tile([128, 512], f32, tag=tag, bufs=1, name=f"pj{g}")
                for ec in range(NE):
                    nc.tensor.matmul(
                        pj[:, :], wqk_s[:, ec, :], xT_g[g][:, ec, :],
                        start=(ec == 0), stop=(ec == NE - 1))
                halves = ((0, 64), (64, 128)) if q_first else ((64, 128),
                                                               (0, 64))
                for eng, (lo, hi) in zip(engs, halves):
                    copy_via(eng, qkT_g[g][lo:hi, :], pj[lo:hi, :])
                    if lo == 0:
                        nc.sync.dma_start(q2_g[g][64:128, :],
                                          qkT_g[g][0:64, :])
                        if after_mirror is not None:
                            after_mirror()

            # ---- v natural: 4 s-tiles (one quad) per staging bank ----
            def emit_vquad(quad, tag, eng="v"):
                vnp = ps.tile([128, 4 * D], f32, tag=tag, bufs=1,
                              name=f"vq{quad}")
                for i in range(4):
                    st = quad * 4 + i
                    g, sub = st // 4, st % 4
                    for ec in range(NE):
                        nc.tensor.matmul(
                            vnp[:, i * D:(i + 1) * D],
                            xT_g[g][:, ec, sub * 128:(sub + 1) * 128],
                            wv_s[:, ec, :],
                            start=(i == 0 and ec == 0),
                            stop=(ec == NE - 1),
                            skip_group_check=True)
                copy_via(eng,
                         v_sb[:, quad * 4:(quad + 1) * 4, 0:D],
                         vnp.rearrange("p (t c) -> p t c", c=D))

            # ---- main loop ----
            out_r = out_d.rearrange("(t p) d -> p t d", p=128)
            acc = {}
            ACTAG = ["ac0", "ac1"]
            pend = []

            def get_acc(gq):
                if gq not in acc:
                    acc[gq] = ps.tile([128, 512], f32, tag=ACTAG[gq % 2],
                                      bufs=1, name=f"acc{gq}")
                return acc[gq]

            def emit_pv(gq, kt, pT, half):
                a = get_acc(gq)
                for j in range(4):
                    nc.tensor.matmul(
                        a[:, j * (D + 1):j * (D + 1) + D + 1],
                        pT[:, half * 512 + j * 128:half * 512 + (j + 1) * 128],
                        v_sb[:, kt, :],
                        start=(kt == 0 and j == 0),
                        stop=(kt == NT - 1),
                        skip_group_check=True,
                    )

            def emit_pair(gq, ktp, pops, split_exp=False):
                sT = ps.tile([128, 1024], f32, tag="a", bufs=2,
                             name=f"sT{gq}_{ktp}")
                for half in range(2):
                    kt = 2 * ktp + half
                    gk = kt // 4
                    ksl = slice((kt % 4) * 128, (kt % 4 + 1) * 128)
                    nc.tensor.matmul(sT[:, half * 512:(half + 1) * 512],
                                     qkT_g[gk][64:128, ksl],
                                     q2_g[gq][64:128, :],
                                     start=True, stop=True)
                pT = ptp.tile([128, 1024], bf16, name="pT")
                if split_exp:
                    nc.scalar.activation(pT[:, 0:512], sT[:, 0:512],
                                         AF.Exp, scale=SCALE)
                    pend.append((gq, 2 * ktp, pT, 0))
                    for _ in range(min(pops, len(pend))):
                        emit_pv(*pend.pop(0))
                    nc.scalar.activation(pT[:, 512:1024], sT[:, 512:1024],
                                         AF.Exp, scale=SCALE)
                    pend.append((gq, 2 * ktp + 1, pT, 1))
                else:
                    nc.scalar.activation(pT[:], sT[:], AF.Exp, scale=SCALE)
                    pend.append((gq, 2 * ktp, pT, 0))
                    pend.append((gq, 2 * ktp + 1, pT, 1))
                    for _ in range(pops):
                        emit_pv(*pend.pop(0))

            # ---- per-group flush: recip + scale + DMA out ----
            lrec = sp.tile([128, NG, 4], f32, name="lrec", tag="lrec")
            out_sbg = {}

            def emit_flush(gq, engs=("v", "v", "v", "v"),
               split_dma=False):
                a = acc[gq]
                av = a[:, 0:4 * (D + 1)].rearrange(
                    "p (t c) -> p t c", c=D + 1)
                out_sbg[gq] = pp.tile([128, 4, D], f32, name=f"out_sb{gq}",
                                      tag=f"out_sb{gq}")
                nc.vector.reciprocal(lrec[:, gq, 0:4], av[:, :, D])
                AF_ = AF
                for jj in range(4):
                    if engs[jj] == "s":
                        nc.scalar.activation(
                            out_sbg[gq][:, jj, :], av[:, jj, 0:D],
                            AF_.Copy, scale=lrec[:, gq, jj:jj + 1])
                    else:
                        nc.vector.tensor_scalar_mul(
                            out_sbg[gq][:, jj, :], av[:, jj, 0:D],
                            lrec[:, gq, jj:jj + 1])
                    if split_dma and jj == 1:
                        nc.sync.dma_start(
                            out_r[:, gq * 4:gq * 4 + 2, :],
                            out_sbg[gq][:, 0:2, :])
                if split_dma:
                    nc.sync.dma_start(out_r[:, gq * 4 + 2:gq * 4 + 4, :],
                                      out_sbg[gq][:, 2:4, :])
                else:
                    nc.sync.dma_start(out_r[:, gq * 4:gq * 4 + 4, :],
                                      out_sbg[gq][:, :, :])

            # ================= emission =================
            emit_warm(N_WARM)
            emit_tp_tile(0, "a", "s", bufs=2)
            emit_tp_tile(1, "a", "v", bufs=2)
            emit_tp_tile(2, "a", "s", bufs=2)
            emit_tp_tile(3, "a", "v", bufs=2)
            def dma_x_pair(pr):
                nc.sync.dma_start(xp[pr][:], x_r[:, 2 * pr:2 * pr + 2, :])

            def after_mirror_g0():
                nc.sync.dma_start(
                    wv_tmp[:], wv_d.rearrange("(p a) d -> p (a d)", a=NE))
                nc.vector.tensor_copy(
                    wv_s[:, :, :],
                    wv_tmp.rearrange("p (a d) -> p a d", a=NE))
                dma_x_pair(4)
                dma_x_pair(5)

            emit_proj(0, "sg0", engs=("s", "v"), q_first=True,
                      after_mirror=after_mirror_g0)
            emit_tp_tile(4, "ac0", "s")
            emit_tp_tile(5, "ac1", "v")
            emit_tp_tile(6, "sg1", "s")
            emit_tp_tile(7, "sg0", "v")
            emit_vquad(0, "sg1", "v")

            fillers = {
                0: [lambda: emit_proj(
                        1, "sg0",
                        after_mirror=lambda: (dma_x_pair(6),
                                              dma_x_pair(7)))],
                1: [lambda: emit_tp_tile(8, "sg1", "v"),
                    lambda: emit_tp_tile(9, "ac0", "v")],
                2: [lambda: emit_tp_tile(10, "sg0", "v"),
                    lambda: emit_tp_tile(11, "ac1", "v")],
                3: [lambda: emit_proj(2, "sg1"),
                    lambda: emit_vquad(1, "sg0", "v")],
                4: [lambda: emit_tp_tile(12, "sg0", "v")],
                5: [lambda: emit_tp_tile(13, "sg0", "v")],
                6: [lambda: emit_tp_tile(14, "sg1", "v")],
                7: [lambda: emit_tp_tile(15, "sg0", "v"),
                    lambda: emit_vquad(2, "sg1", "v"),
                    lambda: emit_proj(3, "sg0")],
                10: [lambda: emit_vquad(3, "sg1", "v")],
            }

            pairs = ([(0, k) for k in range(4)] + [(1, k) for k in range(4)]
                     + [(0, k) for k in range(4, 8)]
                     + [(1, k) for k in range(4, 8)]
                     + [(2, k) for k in range(8)]
                     + [(3, k) for k in range(8)])
            pops_sched = [0, 0, 0] + [2] * 25 + [3, 3, 4, 4]
            flush_at = {14: 0, 18: 1, 26: 2}
            for pi, (gq, ktp) in enumerate(pairs):
                last = (pi == len(pairs) - 1)
                emit_pair(gq, ktp, pops_sched[pi], split_exp=last)
                for f in fillers.get(pi, []):
                    f()
                if pi in flush_at:
                    emit_flush(flush_at[pi])
            while pend:
                emit_pv(*pend.pop(0))
            emit_flush(3, engs=("v", "s", "v", "s"),
                       split_dma=True)

    nc.compile()
    return nc


def kernel(**inputs):
    from concourse.bass_utils import run_bass_kernel_spmd

    x = np.ascontiguousarray(np.asarray(inputs["x"], dtype=np.float32))
    wq = np.ascontiguousarray(np.asarray(inputs["Wq"], dtype=np.float32))
    wk = np.ascontiguousarray(np.asarray(inputs["Wk"], dtype=np.float32))
    wv = np.ascontiguousarray(np.asarray(inputs["Wv"], dtype=np.float32))

    if "nc" not in _CACHE:
        _CACHE["nc"] = _build()
    nc = _CACHE["nc"]

    in_maps = [
        {"x": np.ascontiguousarray(x[b]), "Wq": wq, "Wk": wk, "Wv": wv}
        for b in range(B)
    ]
    res = run_bass_kernel_spmd(nc, in_maps, core_ids=list(range(NCORES)))
    _CACHE["last_results"] = res
    out = np.stack([res.results[b]["out"] for b in range(B)], axis=0)
    return out
